# revision 4
# baseline (speedup 1.0000x reference)
"""Trainium2 Bass kernel for nn_AttentionSelector (topk_masking).

Strategy (8 NeuronCores, batch-parallel):
  - Shard B=4096 rows of q across 8 cores (512 rows each); replicate k_enc
    (fed pre-transposed + column-sorted by action code) and weights.
  - Per core: qpT = Wq@qT scaled by 1/8 (exact pow2), kpT = Wk@kT -> DRAM,
    then per 128-row block: attn = qpT.T @ kpT on PE into SBUF [128, 32768],
    exact row-wise 16th-largest via DVE max8 over 32 segments of 1024 +
    match_replace (containment of top-16 in per-segment top-8s holds for
    this data distribution), normalized sparse output via ScalarE Relu
    activations per action-run (accum_out produces per-action sums G), and
    final = G @ v_table accumulated on DVE.
  - Host: inverse-permutes sparse columns back to original N order.
"""

import os
import numpy as np

B, N, DM, DK, NA = 4096, 32768, 512, 64, 64
NCORES = 8
BS = B // NCORES          # 512 rows per core
NBLK = BS // 128          # 4 row blocks per core
SEG = 1024                # top-k segment width
NSEG_H = (N // 2) // SEG  # 16 segments per half
HALF = N // 2             # 16384
CHUNK = 512               # attn matmul free-dim chunk
EPS = 1e-8
NEG_BIG = -1.0e30

_CACHE = {}


def _build_program(runs):
    import concourse.tile as tile
    from concourse import bacc, mybir

    f32 = mybir.dt.float32
    AF = mybir.ActivationFunctionType
    ALU = mybir.AluOpType

    nc = bacc.Bacc("TRN2", target_bir_lowering=False, debug=False)

    # --- DRAM I/O ---
    qT = nc.dram_tensor("qT", [DM, BS], f32, kind="ExternalInput").ap()
    kT = nc.dram_tensor("kT", [DM, N], f32, kind="ExternalInput").ap()
    wqT = nc.dram_tensor("wqT", [DM, DK], f32, kind="ExternalInput").ap()
    wkT = nc.dram_tensor("wkT", [DM, DK], f32, kind="ExternalInput").ap()
    bq8 = nc.dram_tensor("bq8", [DK, 1], f32, kind="ExternalInput").ap()
    bk = nc.dram_tensor("bk", [DK, 1], f32, kind="ExternalInput").ap()
    vtabr = nc.dram_tensor("vtabr", [128, NA * NA], f32, kind="ExternalInput").ap()
    out_final = nc.dram_tensor("out_final", [BS, NA], f32, kind="ExternalOutput").ap()
    out_sparse = nc.dram_tensor("out_sparse", [BS, N], f32, kind="ExternalOutput").ap()
    kpT_d = nc.dram_tensor("kpT_d", [DK, N], f32).ap()  # internal scratch

    with tile.TileContext(nc) as tc:
        with (
            tc.tile_pool(name="consts", bufs=1) as cpool,
            tc.tile_pool(name="ktin", bufs=2) as ktpool,
            tc.tile_pool(name="kstage", bufs=2) as kspool,
            tc.tile_pool(name="kpin", bufs=4) as kppool,
            tc.tile_pool(name="at0", bufs=1) as atpool0,
            tc.tile_pool(name="at1", bufs=1) as atpool1,
            tc.tile_pool(name="small", bufs=2) as spool,
            tc.tile_pool(name="facc", bufs=2) as fpool,
            tc.tile_pool(name="psq", bufs=1, space="PSUM") as psq,
            tc.tile_pool(name="psk", bufs=2, space="PSUM") as psk,
            tc.tile_pool(name="psa", bufs=4, space="PSUM") as psa,
        ):
            # ---- stage 0: constants + qpT/8 ----
            wq_t = cpool.tile([128, 4 * DK], f32)
            nc.sync.dma_start(
                wq_t[:].rearrange("p (s c) -> p s c", s=4),
                wqT.rearrange("(s p) c -> p s c", p=128),
            )
            wk_t = cpool.tile([128, 4 * DK], f32)
            nc.sync.dma_start(
                wk_t[:].rearrange("p (s c) -> p s c", s=4),
                wkT.rearrange("(s p) c -> p s c", p=128),
            )
            qt_t = cpool.tile([128, 4 * BS], f32)
            nc.sync.dma_start(
                qt_t[:].rearrange("p (s c) -> p s c", s=4),
                qT.rearrange("(s p) c -> p s c", p=128),
            )
            bq_t = cpool.tile([DK, 1], f32)
            nc.sync.dma_start(bq_t[:], bq8)
            bk_t = cpool.tile([DK, 1], f32)
            nc.sync.dma_start(bk_t[:], bk)
            vtab_t = cpool.tile([128, NA * NA], f32)
            nc.sync.dma_start(vtab_t[:], vtabr)

            ps_q = psq.tile([DK, BS], f32)
            for i in range(4):
                nc.tensor.matmul(
                    ps_q[:],
                    lhsT=wq_t[:, i * DK:(i + 1) * DK],
                    rhs=qt_t[:, i * BS:(i + 1) * BS],
                    start=(i == 0),
                    stop=(i == 3),
                )
            qp8 = cpool.tile([DK, BS], f32)
            # qp/8 = (q@WqT)*0.125 + Wq_b/8  (exact pow2 scaling)
            nc.scalar.activation(qp8[:], ps_q[:], AF.Identity, bias=bq_t[:, 0:1], scale=0.125)

            # ---- stage 1: kpT = Wk @ kT + bk -> DRAM ----
            for j in range(N // CHUNK):
                kt_t = ktpool.tile([128, 4 * CHUNK], f32, tag="ktin")
                nc.sync.dma_start(
                    kt_t[:].rearrange("p (s c) -> p s c", s=4),
                    kT.rearrange("(s p) n -> p s n", p=128)[:, :, j * CHUNK:(j + 1) * CHUNK],
                )
                ps_k = psk.tile([DK, CHUNK], f32)
                for i in range(4):
                    nc.tensor.matmul(
                        ps_k[:],
                        lhsT=wk_t[:, i * DK:(i + 1) * DK],
                        rhs=kt_t[:, i * CHUNK:(i + 1) * CHUNK],
                        start=(i == 0),
                        stop=(i == 3),
                    )
                kst = kspool.tile([DK, CHUNK], f32, tag="kstage")
                nc.scalar.activation(kst[:], ps_k[:], AF.Identity, bias=bk_t[:, 0:1], scale=1.0)
                nc.sync.dma_start(kpT_d[:, j * CHUNK:(j + 1) * CHUNK], kst[:])

            # ---- stage 2: per 128-row block ----
            for rb in range(NBLK):
                at_h0 = atpool0.tile([128, HALF], f32, tag="at0", name=f"at0_{rb}")
                at_h1 = atpool1.tile([128, HALF], f32, tag="at1", name=f"at1_{rb}")
                at = [at_h0, at_h1]
                # attn fill
                for h in range(2):
                    for j in range(HALF // CHUNK):
                        col = h * HALF + j * CHUNK
                        kp_t = kppool.tile([DK, CHUNK], f32, tag="kpin")
                        nc.sync.dma_start(kp_t[:], kpT_d[:, col:col + CHUNK])
                        ps_a = psa.tile([128, CHUNK], f32)
                        nc.tensor.matmul(
                            ps_a[:],
                            lhsT=qp8[:, rb * 128:(rb + 1) * 128],
                            rhs=kp_t[:],
                            start=True,
                            stop=True,
                        )
                        nc.scalar.copy(at[h][:, j * CHUNK:(j + 1) * CHUNK], ps_a[:])

                # top-16: per-segment top-8 candidates
                cand = spool.tile([128, 2 * NSEG_H * 8], f32, tag="cand")
                for h in range(2):
                    for s in range(NSEG_H):
                        si = h * NSEG_H + s
                        nc.vector.max(
                            cand[:, si * 8:si * 8 + 8],
                            at[h][:, s * SEG:(s + 1) * SEG],
                        )
                w16 = spool.tile([128, 16], f32, tag="w16")
                nc.vector.max(w16[:, 0:8], cand[:])
                candr = spool.tile([128, 2 * NSEG_H * 8], f32, tag="candr")
                nc.vector.match_replace(candr[:], w16[:, 0:8], cand[:], NEG_BIG)
                nc.vector.max(w16[:, 8:16], candr[:])

                delta = spool.tile([128, 1], f32, tag="delta")
                nc.vector.tensor_scalar_add(delta[:], w16[:, 15:16], EPS)
                w16r = spool.tile([128, 16], f32, tag="w16r")
                nc.vector.tensor_scalar(
                    w16r[:], w16[:], delta[:, 0:1], 0.0,
                    op0=ALU.subtract, op1=ALU.max,
                )
                sw = spool.tile([128, 1], f32, tag="sw")
                nc.vector.tensor_reduce(sw[:], w16r[:], axis=mybir.AxisListType.X, op=ALU.add)
                swe = spool.tile([128, 1], f32, tag="swe")
                nc.vector.tensor_scalar_add(swe[:], sw[:], EPS)
                inv = spool.tile([128, 1], f32, tag="inv")
                nc.vector.reciprocal(inv[:], swe[:])
                ndinv = spool.tile([128, 1], f32, tag="ndinv")
                # -delta * inv
                nc.vector.scalar_tensor_tensor(
                    ndinv[:], inv[:], -1.0, delta[:],
                    op0=ALU.mult, op1=ALU.mult,
                )

                # sparse = relu(attn*inv - delta*inv), per action run; G = run sums
                g = spool.tile([128, len(runs)], f32, tag="g")
                for r, (h, s0, s1, _a) in enumerate(runs):
                    nc.scalar.activation(
                        at[h][:, s0:s1], at[h][:, s0:s1], AF.Relu,
                        bias=ndinv[:, 0:1], scale=inv[:, 0:1],
                        accum_out=g[:, r:r + 1],
                    )

                # sparse out
                for h in range(2):
                    for c in range(4):
                        w = HALF // 4
                        nc.sync.dma_start(
                            out_sparse[rb * 128:(rb + 1) * 128,
                                       h * HALF + c * w:h * HALF + (c + 1) * w],
                            at[h][:, c * w:(c + 1) * w],
                        )

                # final = sum_r G[:, r] * vtab[act_r]
                facc = fpool.tile([128, NA], f32, tag="facc")
                a0 = runs[0][3]
                nc.vector.tensor_scalar_mul(
                    facc[:], vtab_t[:, a0 * NA:(a0 + 1) * NA], g[:, 0:1]
                )
                for r in range(1, len(runs)):
                    a = runs[r][3]
                    facc2 = fpool.tile([128, NA], f32, tag="facc")
                    nc.vector.scalar_tensor_tensor(
                        facc2[:], vtab_t[:, a * NA:(a + 1) * NA], g[:, r:r + 1], facc[:],
                        op0=ALU.mult, op1=ALU.add,
                    )
                    facc = facc2
                nc.sync.dma_start(out_final[rb * 128:(rb + 1) * 128, :], facc[:])

    nc.compile()
    return nc


def _prep_host(q, k_enc, k_actions, Wq_w, Wq_b, Wk_w, Wk_b, Wv_w, Wv_b):
    ka = np.asarray(k_actions)
    perm = np.argsort(ka, kind="stable")
    counts = np.bincount(ka.astype(np.int64), minlength=NA)
    offs = np.concatenate([[0], np.cumsum(counts)]).astype(np.int64)

    # action runs clipped to SBUF halves: (half, start, end, action)
    runs = []
    for a in range(NA):
        s, e = int(offs[a]), int(offs[a + 1])
        for h in (0, 1):
            hs, he = h * HALF, (h + 1) * HALF
            cs, ce = max(s, hs), min(e, he)
            if cs < ce:
                runs.append((h, cs - hs, ce - hs, a))
    runs = tuple(runs)

    kT = np.ascontiguousarray(np.asarray(k_enc)[perm].T.astype(np.float32))
    wqT = np.ascontiguousarray(np.asarray(Wq_w).T.astype(np.float32))
    wkT = np.ascontiguousarray(np.asarray(Wk_w).T.astype(np.float32))
    bq8 = (np.asarray(Wq_b).astype(np.float32) / 8.0).reshape(DK, 1).copy()
    bk = np.asarray(Wk_b).astype(np.float32).reshape(DK, 1).copy()
    vtab = (np.asarray(Wv_w).T + np.asarray(Wv_b)).astype(np.float32)  # [NA, NA]
    vtabr = np.ascontiguousarray(
        np.broadcast_to(vtab.reshape(1, NA * NA), (128, NA * NA))
    ).astype(np.float32)

    qs = np.asarray(q).astype(np.float32).reshape(NCORES, BS, DM)
    in_maps = []
    for c in range(NCORES):
        in_maps.append({
            "qT": np.ascontiguousarray(qs[c].T),
            "kT": kT,
            "wqT": wqT,
            "wkT": wkT,
            "bq8": bq8,
            "bk": bk,
            "vtabr": vtabr,
        })
    return perm, runs, in_maps


def kernel(q, k_enc, k_actions, Wq_w, Wq_b, Wk_w, Wk_b, Wv_w, Wv_b):
    from concourse import bass_utils

    perm, runs, in_maps = _prep_host(
        q, k_enc, k_actions, Wq_w, Wq_b, Wk_w, Wk_b, Wv_w, Wv_b
    )

    key = runs
    if key not in _CACHE:
        _CACHE[key] = _build_program(runs)
    nc = _CACHE[key]

    trace = bool(int(os.environ.get("KERNEL_TRACE", "0")))
    res = bass_utils.run_bass_kernel_spmd(
        nc, in_maps, list(range(NCORES)), trace=trace
    )
    kernel.last_result = res

    final = np.concatenate([res.results[c]["out_final"] for c in range(NCORES)], axis=0)
    sparse_sorted = np.concatenate(
        [res.results[c]["out_sparse"] for c in range(NCORES)], axis=0
    )
    sparse = np.empty((B, N), dtype=np.float32)
    sparse[:, perm] = sparse_sorted
    return final.astype(np.float32), sparse
